# revision 2
# baseline (speedup 1.0000x reference)
"""AdaptiveRankLinear on 8 TRN2 NeuronCores.

y[b,t,o] = sum_i x[b,t,i] * W[o,i] + bias[o],  W = U @ (diag(S) @ Vt)

Sharding: pure data-parallel over batch (B=8 == n_cores); U/S/Vt/bias
replicated. Per core: y_b = (x_b @ Vts^T) @ U^T + bias via the rank-256
bottleneck — 2 chained matmuls instead of materializing the 4096x4096 W.

v2 vs v1:
  - x stored as fp8 e3m4 (1 byte): halves x HBM traffic (8.4MB vs 16.8MB
    per core). mm1 runs mixed-dtype (bf16 stationary Vts^T, e3m4 moving
    x) at full rate; one e3m4 quantization hit costs ~1.2e-2 rel err vs
    the 2e-2 gate (measured host-side). Less DMA also means less chip
    power -> less P0 clock throttle (8-core PE sustains ~2.0GHz vs
    2.4GHz single-core; DMA/descriptor power is the only lever we have).
  - host packs x/Vts^T into per-(chunk,group) contiguous blocks so every
    load is descriptor-friendly (4-8KB per-partition lines).
  - PE program order: chunk-0 mm1 first (no memset/bias dependency), so
    the PE starts as soon as the first 0.5MB lands; bias broadcast
    matmuls run after chunk-0 mm1.
  - mm2(c) and mm1(c+1) are emitted interleaved (m-block granularity) so
    DVE evacuation work spreads over the whole timeline instead of
    cramming into mm2-only windows.
  - y stores ride the scalar-engine HWDGE ring (separate from the sync
    load ring); last chunk splits evacuation between ScalarE and DVE and
    stores per-1KB-slice to shorten the tail.
Compute: bf16 matmuls f32 PSUM accumulate, bf16 output (host casts back
to f32). rel err ~1.2e-2 vs the 2e-2 gate.
"""

import numpy as np
import ml_dtypes

B, T, IN, OUT, RANK = 8, 2048, 4096, 4096, 256
N_CORES = 8
P = 128
TC = 512               # T chunk (psum bank = 512 f32)
NCHUNK = T // TC       # 4
NIT = IN // P          # 32 contraction tiles for mm1
NRT = RANK // P        # 2 rank tiles
OC = 512               # matmul free-dim max
MT = TC // P           # 4 T-tiles per chunk
NG = 4                 # x/vtst load groups per chunk
GN = NIT // NG         # IN tiles per load group (8)

BF16 = ml_dtypes.bfloat16
E3M4 = ml_dtypes.float8_e3m4

_CACHE = {}


def _build():
    import concourse.bacc as bacc
    import concourse.bass as bass
    import concourse.tile as tile
    from concourse import mybir

    f32 = mybir.dt.float32
    bf16 = mybir.dt.bfloat16
    f8e3 = mybir.dt.float8e3

    nc = bacc.Bacc("TRN2", target_bir_lowering=False, debug=False,
                   num_devices=N_CORES)
    # packed layouts (host-prepped): per (chunk, group) x block is
    # [P, GN*TC] with contiguous per-partition rows; per group vtst block
    # is [P, GN*RANK].
    xp = nc.dram_tensor("xp", [NCHUNK * NG * P, GN * TC], f8e3,
                        kind="ExternalInput")
    vp = nc.dram_tensor("vp", [NG * P, GN * RANK], bf16,
                        kind="ExternalInput")
    ut = nc.dram_tensor("ut", [RANK, OUT], bf16, kind="ExternalInput")
    biasb = nc.dram_tensor("biasb", [1, OUT], bf16, kind="ExternalInput")
    out = nc.dram_tensor("out", [T, OUT], bf16, kind="ExternalOutput")

    with tile.TileContext(nc) as tc:
        with (
            tc.tile_pool(name="weights", bufs=1) as wpool,
            tc.tile_pool(name="xin", bufs=12) as xpool,
            tc.tile_pool(name="tt", bufs=3) as tpool,
            tc.tile_pool(name="yout", bufs=4) as ypool,
            tc.tile_pool(name="pt", bufs=1, space=bass.MemorySpace.PSUM) as ptp,
            tc.tile_pool(name="py", bufs=3, space=bass.MemorySpace.PSUM) as pyp,
        ):
            ones_t = wpool.tile([1, P], bf16, tag="ones")
            nc.vector.memset(ones_t[:], 1.0)

            def load_x_group(c, g, halves=1):
                xg = xpool.tile([P, GN * TC], f8e3, tag="xg",
                                name=f"xg_{c}_{g}")
                r0 = (c * NG + g) * P
                w = GN * TC // halves
                for hh in range(halves):
                    nc.sync.dma_start(xg[:, hh * w:(hh + 1) * w],
                                      xp[r0:r0 + P, hh * w:(hh + 1) * w])
                return xg

            # ---- all loads on the sync queue in need-order ----
            # DMA completion on a queue is FIFO, so bytes queued ahead of a
            # load ARE its latency: interleave vtst groups with chunk-0 x
            # groups, g=0 split in halves so the first matmul only waits
            # ~0.5MB.
            vtst_g = []
            xc = {}
            for g in range(NG):
                halves = 2 if g == 0 else 1
                vw = wpool.tile([P, GN * RANK], bf16, tag=f"vtst{g}",
                                name=f"vtst{g}")
                wv = GN * RANK // halves
                for hh in range(halves):
                    nc.sync.dma_start(vw[:, hh * wv:(hh + 1) * wv],
                                      vp[g * P:(g + 1) * P,
                                         hh * wv:(hh + 1) * wv])
                    if g == 0 and hh == 0:
                        xc[(0, 0)] = load_x_group(0, 0, halves=2)
                vtst_g.append(vw)
                if g > 0:
                    xc[(0, g)] = load_x_group(0, g)

            # ut/bias next on the same queue: needed by mm2 of chunk 0,
            # ~30us after the first matmul.
            ut_sb = []
            for j in range(NRT):
                u = wpool.tile([P, OUT], bf16, tag=f"ut{j}")
                nc.sync.dma_start(u[:], ut[j * P:(j + 1) * P, :])
                ut_sb.append(u)

            bias_row = wpool.tile([1, OUT], bf16, tag="bias_row")
            nc.sync.dma_start(bias_row[:], biasb[:, :])

            # remaining x chunks, in consumption order
            for c in range(1, NCHUNK):
                for g in range(NG):
                    xc[(c, g)] = load_x_group(c, g)

            bias_sb = wpool.tile([P, OUT], bf16, tag="bias")
            tts = {}

            def emit_mm1(c, n0, n1):
                if n0 == 0:
                    tts[c] = {}
                    tts[c]["pt"] = [
                        ptp.tile([P, TC], f32, tag=f"pt{j}", name=f"pt{j}_{c}")
                        for j in range(NRT)]
                pt = tts[c]["pt"]
                for n in range(n0, n1):
                    g, nl = divmod(n, GN)
                    for j in range(NRT):
                        nc.tensor.matmul(
                            pt[j][:],
                            vtst_g[g][:, nl * RANK + j * P:
                                      nl * RANK + (j + 1) * P],
                            xc[(c, g)][:, nl * TC:(nl + 1) * TC],
                            start=(n == 0), stop=(n == NIT - 1))
                if n1 == NIT:
                    # evacuate pt -> tt (bf16) on DVE; mm2 reads tt
                    tts[c]["tt"] = []
                    for j in range(NRT):
                        ttj = tpool.tile([P, TC], bf16, tag=f"tt{j}",
                                         name=f"tt{j}_{c}")
                        nc.vector.tensor_copy(ttj[:], pt[j][:])
                        tts[c]["tt"].append(ttj)

            def emit_bias_bcast():
                # partition-broadcast bias with K=1 matmuls against ones
                for q in range(OUT // 1024):
                    pb = pyp.tile([P, 1024], f32, tag="py", name=f"pb{q}")
                    for h in range(2):
                        o0 = q * 1024 + h * OC
                        nc.tensor.matmul(pb[:, h * OC:(h + 1) * OC],
                                         ones_t[:, :],
                                         bias_row[:, o0:o0 + OC],
                                         start=True, stop=True)
                    nc.vector.tensor_copy(bias_sb[:, q * 1024:(q + 1) * 1024],
                                          pb[:])

            def emit_mm2_block(c, m):
                tt = tts[c]["tt"]
                last_c = c == NCHUNK - 1
                last_m = last_c and m == MT - 1
                row = (c * MT + m) * P
                y = ypool.tile([P, OUT], bf16, tag="y")
                for oh in range(OUT // 1024):
                    py = pyp.tile([P, 1024], f32, tag="py")
                    for j in range(NRT):
                        for oo in range(2):
                            o0 = oh * 1024 + oo * OC
                            nc.tensor.matmul(
                                py[:, oo * OC:(oo + 1) * OC],
                                tt[j][:, m * P:(m + 1) * P],
                                ut_sb[j][:, o0:o0 + OC],
                                start=(j == 0), stop=(j == NRT - 1))
                    ys = y[:, oh * 1024:(oh + 1) * 1024]
                    bs = bias_sb[:, oh * 1024:(oh + 1) * 1024]
                    if last_c and (m * 4 + oh) % 2 == 0:
                        # final chunk has no following mm1 to absorb the
                        # DVE backlog; split evacuation with ScalarE
                        nc.scalar.copy(ys, py[:])
                        nc.vector.tensor_add(ys, ys, bs)
                    else:
                        nc.vector.tensor_add(ys, py[:], bs)
                    if last_m:
                        # final tile: store per-oh so the last bytes leave
                        # right after their ADD (shorter tail)
                        nc.scalar.dma_start(
                            out[row:row + P, oh * 1024:(oh + 1) * 1024],
                            y[:, oh * 1024:(oh + 1) * 1024])
                if not last_m:
                    nc.scalar.dma_start(out[row:row + P, :], y[:])

            # ---- PE program ----
            emit_mm1(0, 0, NIT)
            emit_bias_bcast()
            seg = NIT // MT  # mm1 rows per interleave segment (8)
            for c in range(NCHUNK):
                for m in range(MT):
                    emit_mm2_block(c, m)
                    if c + 1 < NCHUNK:
                        emit_mm1(c + 1, m * seg, (m + 1) * seg)

    nc.compile()
    return nc


def _prep_in_maps(x, U, S, Vt, bias):
    x = np.asarray(x, dtype=np.float32)
    U = np.asarray(U, dtype=np.float32)
    S = np.asarray(S, dtype=np.float32)
    Vt = np.asarray(Vt, dtype=np.float32)
    bias = np.asarray(bias, dtype=np.float32)

    vtstT = np.ascontiguousarray((S[:, None] * Vt).T).astype(BF16)  # [IN,R]
    v4 = np.asarray(vtstT).reshape(NIT, P, RANK)
    vp_np = np.concatenate(
        [v4[g * GN:(g + 1) * GN].transpose(1, 0, 2).reshape(P, GN * RANK)
         for g in range(NG)], axis=0)                              # [NG*P, GN*R]
    ut_np = np.ascontiguousarray(U.T).astype(BF16)                 # [R,OUT]
    biasb_np = np.ascontiguousarray(bias[None, :]).astype(BF16)    # [1,OUT]

    in_maps = []
    for c in range(N_CORES):
        xT = np.ascontiguousarray(x[c].T).astype(E3M4)             # [IN,T]
        x4 = xT.reshape(NIT, P, T)
        blocks = []
        for cc in range(NCHUNK):
            for g in range(NG):
                blocks.append(
                    x4[g * GN:(g + 1) * GN, :, cc * TC:(cc + 1) * TC]
                    .transpose(1, 0, 2).reshape(P, GN * TC))
        xp_np = np.concatenate(blocks, axis=0)        # [NCHUNK*NG*P, GN*TC]
        in_maps.append({"xp": xp_np, "vp": vp_np, "ut": ut_np,
                        "biasb": biasb_np})
    return in_maps


def _run(inputs, trace=False, trace_kwargs=None):
    import concourse.bass_utils as bass_utils
    if trace:
        bass_utils.upload_artifacts = lambda tmpdir: tmpdir
    if "nc" not in _CACHE:
        _CACHE["nc"] = _build()
    nc = _CACHE["nc"]
    in_maps = _prep_in_maps(**inputs)
    res = bass_utils.run_bass_kernel_spmd(
        nc, in_maps, core_ids=list(range(N_CORES)), trace=trace,
        **(trace_kwargs or {}))
    y = np.stack([res.results[c]["out"] for c in range(N_CORES)],
                 axis=0).astype(np.float32)
    return y, res


def kernel(**inputs) -> np.ndarray:
    y, _ = _run(inputs, trace=False)
    return y
